# revision 84
# baseline (speedup 1.0000x reference)
"""Trainium2 Bass kernel for CascadedNN (dense_mlp).

Math (per batch row x of dim 256):
  f  = relu(x @ W1 + b1)           # 512
  f  = relu(f @ W2 + b2)           # 256
  first = sigmoid(f @ Wf + bf)
  a_t = f @ Wc[t,:256] + bc[t]     (t = 0..62)
  p_0 = first;  p_{t+1} = max(sigmoid(a_t + w_t * p_t), p_t),  w_t = Wc[t,256]
  out = [p_0, ..., p_63]           # [B, 64]

Strategy: pure data parallel over 8 cores (8192 rows each), fp16 GEMMs
with fp32 PSUM accumulation, feature-major L1/L2 (x pre-transposed on
the host). The head runs batch-major - each [128 feat, 128 batch] f2
block is the stationary operand against Wcat [256, 64], landing
[128 batch, 64 steps] tiles in PSUM with t along the free dim.

The 63-step cascade is run as two fixed-point passes (contraction
factor q = 0.25*max|w| ~ 0.04, error <= 0.5*q^2 < 1e-3):
  pass A: p~_t = cummax_{s<=t} sigmoid(ZA_s), ZA = head logits with
          bc + 0.5*w + C folded in (prev ~= 0.5 linearization).
  pass B: s_t = sigmoid(ZA_t + w_t*(p~_{t-1}-0.5)); out = cummax(s_t)

Key tricks vs a naive lowering:
  - bias (bc + 0.5w + C) folded into the head GEMM as a K=2 rank-1
    matmul with fp16 hi/lo rows: PSUM holds finished pass-A logits.
  - sigmoid is monotone, so cummax(sigmoid(z)) = sigmoid(cummax(z)):
    pass A scans the raw PSUM logits (fp32!) and sigmoids the scan.
  - the +C (=16) offset keeps all logits positive, which lets ONE
    [128,512] tensor_tensor_scan with op0=mult on a 0/1 column mask
    handle all eight 64-step groups (state resets to max(0, z) at each
    group boundary).
  - evacuations are [128,1024] double-bank ops split between ACT and
    DVE (gpsimd has no PSUM port); gpsimd runs the SBUF-only head ops
    (scalar_tensor_tensor, copies, pass-B scan) and the output DMA.
  - x loads are hoisted upfront on the sync HWDGE ring; output stores
    ride the gpsimd SWDGE ring, so stores never head-of-line block
    loads.
"""

import numpy as np
import ml_dtypes
from contextlib import ExitStack

import concourse.bacc as bacc
import concourse.bass as bass
import concourse.mybir as mybir
from concourse import tile
from concourse.bass_utils import run_bass_kernel_spmd

# Both Relu (L1/L2 evac) and Sigmoid (cascade) live in the
# "sigmoid_and_others" activation table. Left alone, walrus assigns Relu
# to the first table containing it ("exp_and_others") and Sigmoid to
# this one, forcing two 1.3us table reloads per loop iteration on the
# ACT engine. Empty out every other table so all activations resolve to
# the shared one (dict order, hence act_func_set_id, is preserved).
_ORIG_GAT = bacc.get_activation_tables


def _gat_one_table(arch):
    tabs = _ORIG_GAT(arch)
    return {name: (funcs if name == "sigmoid_and_others" else set())
            for name, funcs in tabs.items()}


bacc.get_activation_tables = _gat_one_table

FP16 = mybir.dt.float16
FP8 = mybir.dt.float8e4
F32 = mybir.dt.float32
AF = mybir.ActivationFunctionType
OP = mybir.AluOpType
PM = mybir.MatmulPerfMode

B, D, H1, H2, T = 65536, 256, 512, 256, 64
NCORES = 8
BL = B // NCORES            # 8192 rows per core
NCHUNK = 4
CB = BL // NCHUNK           # 2048 rows per chunk
NPAIR = CB // 1024          # [128,1024] psum pairs per chunk row-block
NBANK = 2                   # head psum banks per chunk (each 8 j-tiles)
COFF = 16.0                 # logit offset keeping head logits positive

_CACHE = {}


def _build(bench_nrep=0, rev="r1", evac_pat="DAADADAADADA", xup=True,
           do_post=True, mm_only=False, l1fp8=False, post_delay=True,
           za_copy=True, tail_split=True, startup_opt=True, nchunk=NCHUNK,
           xring=False):
    # chunking geometry is local so nchunk can vary per build (shadows
    # the module-level defaults inside this function)
    NCHUNK = nchunk
    CB = BL // nchunk
    NPAIR = CB // 1024
    NBANK = CB // 1024
    """evac_pat: engine rotation for L1/L2 psum pair evacuation
    (A=ACT, D=DVE), consumed round-robin, 12 pairs per chunk.
    do_post=False: bench-only, dump raw head logits (skip the cascade).
    mm_only=True: bench-only, pure PE throughput (constant operands).
    l1fp8: L1 via fp8e4 DoubleRow matmuls - x quantized once, W1 as an
    exact hi+lo fp8 pair (two matmuls), contraction 256 per pass."""
    nc = bacc.Bacc("TRN2", target_bir_lowering=False, debug=False,
                   num_devices=NCORES)
    # unique per-variant dummy input: defeats NEFF/executable cache
    # collisions between structurally-different builds with identical I/O
    vtag = nc.dram_tensor(
        f"vtag_r{bench_nrep}e{evac_pat}x{int(xup)}q{int(do_post)}"
        f"m{int(mm_only)}f{int(l1fp8)}p{int(post_delay)}z{int(za_copy)}"
        f"t{int(tail_split)}s{int(startup_opt)}n{nchunk}r{int(xring)}"
        f"v{rev}",
        [1, 1], F32, kind="ExternalInput")

    if l1fp8:
        x8t = nc.dram_tensor("x8t", [128, 2, BL], FP8, kind="ExternalInput")
        w18 = nc.dram_tensor("w18", [2, 128, 2, H1], FP8,
                             kind="ExternalInput")
    else:
        xt = nc.dram_tensor("xt", [2, 128, BL], FP16, kind="ExternalInput")
    # constants are consolidated into few tensors: each DMA costs ~1us of
    # serial HWDGE ring time regardless of size, and the old 15-load
    # startup stalled L2 ~6us waiting for W2
    w1 = nc.dram_tensor("w1", [128, 2 * H1], FP16, kind="ExternalInput")
    w2 = nc.dram_tensor("w2", [128, 4 * H2], FP16, kind="ExternalInput")
    wcat = nc.dram_tensor("wcat", [128, 2 * T], FP16, kind="ExternalInput")
    # [128, 8] f32: cols 0-3 b1 m-tiles, 4-5 b2 m-tiles, 6 = -C, 7 = -C/2
    cst32 = nc.dram_tensor("cst32", [128, 8], F32, kind="ExternalInput")
    # [128, 1024] fp16: cols 0-511 scan mask, 512-1023 halved w row
    mwg = nc.dram_tensor("mwg", [128, 1024], FP16, kind="ExternalInput")
    # [2, 640] fp16: cols 0-511 head-bias hi/lo rows, 512-639 ones
    bro = nc.dram_tensor("bro", [2, 640], FP16, kind="ExternalInput")
    out = nc.dram_tensor("out", [BL, T], FP16, kind="ExternalOutput")

    with tile.TileContext(nc) as tc, ExitStack() as ctx:
        wpool = ctx.enter_context(tc.tile_pool(name="wts", bufs=1))
        xpool = ctx.enter_context(tc.tile_pool(name="xin", bufs=1))
        f1pool = ctx.enter_context(tc.tile_pool(name="f1", bufs=2))
        f2pool = ctx.enter_context(tc.tile_pool(name="f2", bufs=2))
        hpool = ctx.enter_context(tc.tile_pool(name="hd", bufs=3))
        pspool = ctx.enter_context(
            tc.tile_pool(name="ps", bufs=3, space=bass.MemorySpace.PSUM))

        # resident weights / constants (few big tiles; per-operand views)
        if l1fp8:
            w18sb = [wpool.tile([128, 2, H1], FP8, name=f"w18_{w}",
                                tag=f"w18_{w}") for w in range(2)]
        else:
            w1all = wpool.tile([128, 2 * H1], FP16, name="w1a", tag="w1a")
            w1sb = [w1all[:, k * H1:(k + 1) * H1] for k in range(2)]
        w2all = wpool.tile([128, 4 * H2], FP16, name="w2a", tag="w2a")
        w2sb = [w2all[:, k * H2:(k + 1) * H2] for k in range(4)]
        wcall = wpool.tile([128, 2 * T], FP16, name="wca", tag="wca")
        wcsb = [wcall[:, k * T:(k + 1) * T] for k in range(2)]
        cstsb = wpool.tile([128, 8], F32, name="cst", tag="cst")
        b1sb = [cstsb[:, m:m + 1] for m in range(4)]
        b2sb = [cstsb[:, 4 + m:5 + m] for m in range(2)]
        ncsb = cstsb[:, 6:7]
        nchsb = cstsb[:, 7:8]
        mwgsb = wpool.tile([128, 1024], FP16, name="mwg", tag="mwg")
        masksb = mwgsb[:, 0:512]
        wgsb = mwgsb[:, 512:1024]
        brosb = wpool.tile([2, 640], FP16, name="bro", tag="bro")
        brsb = brosb[:, 0:512]
        onsb = brosb[:, 512:640]
        vtsb = wpool.tile([1, 1], F32, name="vt", tag="vt")

        # Resident loads ride the two HWDGE rings (Pool SWDGE costs ~1us
        # of engine-side descriptor generation per transfer). sync ring:
        # w1 first (needed by the very first matmul), then x chunk
        # loads. scalar ring: vtag first (the act-table warmup waits on
        # it and would otherwise head-of-line block ACT's queue), then
        # first-use order.
        if l1fp8:
            for w in range(2):
                nc.sync.dma_start(w18sb[w][:], w18[w])
        else:
            nc.sync.dma_start(w1all[:], w1[:])
        # xring: late-needed residents ride the Pool SWDGE ring (first
        # use is L2 at ~7us) so the scalar HWDGE ring is free to carry
        # chunk-0's k1 x quarters from t=0
        rring = nc.gpsimd if xring else nc.scalar
        rring.dma_start(vtsb[:], vtag[:])
        rring.dma_start(cstsb[:], cst32[:])
        rring.dma_start(w2all[:], w2[:])
        rring.dma_start(wcall[:], wcat[:])
        rring.dma_start(brosb[:], bro[:])
        rring.dma_start(mwgsb[:], mwg[:])

        wg3 = wgsb[:].rearrange("p (g t) -> p g t", t=T)

        # pre-loop dummy activation: puts the (single) act table load on
        # the loop-preheader path so the fixpoint pass hoists it out of
        # the For_i body.
        dummy = wpool.tile([1, 1], F32, name="du", tag="du")
        nc.scalar.activation(dummy[:], vtsb[:], AF.Sigmoid)

        # output view: out[f*128 + p, t] <- OUT[p, f_within, t]
        ov = out[:].rearrange("(f p) t -> p f t", p=128)

        if mm_only:
            # pure PE floor: constant operands loaded once, no evac deps
            xmm = [xpool.tile([128, CB], FP16, name=f"xm{k}", tag=f"xm{k}")
                   for k in range(2)]
            for k in range(2):
                nc.sync.dma_start(xmm[k][:], xt[k][:, 0:CB])
            f1mm = [f1pool.tile([128, CB], FP16, name=f"fm1_{m}",
                                tag=f"fm1_{m}", bufs=1) for m in range(4)]
            f2mm = [f2pool.tile([128, CB], FP16, name=f"fm2_{m}",
                                tag=f"fm2_{m}", bufs=1) for m in range(2)]
            for m in range(4):
                nc.gpsimd.memset(f1mm[m][:], 0.25)
            for m in range(2):
                nc.gpsimd.memset(f2mm[m][:], 0.25)

        loop = tc.For_i(0, bench_nrep, 1) if bench_nrep else None
        if loop is not None:
            loop.__enter__()

        if mm_only:
            for c in range(NCHUNK):
                for m in range(4):
                    prs = [pspool.tile([128, 1024], F32, name="ps",
                                       tag="ps", bufs=3)
                           for _ in range(NPAIR)]
                    for k in range(2):
                        for nb in range(2 * NPAIR):
                            nc.tensor.matmul(
                                prs[nb // 2][:, bass.ts(nb % 2, 512)],
                                w1sb[k][:, bass.ts(m, 128)],
                                xmm[k][:, bass.ts(nb, 512)],
                                start=(k == 0), stop=(k == 1))
                for m in range(2):
                    prs = [pspool.tile([128, 1024], F32, name="ps",
                                       tag="ps", bufs=3)
                           for _ in range(NPAIR)]
                    for k in range(4):
                        for nb in range(2 * NPAIR):
                            nc.tensor.matmul(
                                prs[nb // 2][:, bass.ts(nb % 2, 512)],
                                w2sb[k][:, bass.ts(m, 128)],
                                f1mm[k][:, bass.ts(nb, 512)],
                                start=(k == 0), stop=(k == 3))
                for bi in range(NBANK):
                    psw = pspool.tile([128, 512], F32, name="psw",
                                      tag="psh", bufs=2)
                    nc.tensor.matmul(psw[:], onsb[:], brsb[:],
                                     start=True, stop=False,
                                     skip_group_check=True)
                    for j8 in range(8):
                        for k in range(2):
                            nc.tensor.matmul(
                                psw[:, bass.ts(j8, T)],
                                f2mm[k][:, bass.ts(bi * 8 + j8, 128)],
                                wcsb[k][:], start=False, stop=(k == 1),
                                skip_group_check=True)
                    # anchor: tiny evac + store so the chunk has a sink
                    OUTa = hpool.tile([128, 512], FP16, name=f"oa_{bi}",
                                      tag=f"ou_{bi}")
                    nc.vector.tensor_scalar(OUTa[:], psw[:], 0.0, 0.0,
                                            OP.add, OP.max)
                    o3 = OUTa[:].rearrange("p (g t) -> p g t", t=T)
                    fbase = c * (CB // 128) + bi * 8
                    nc.gpsimd.dma_start(ov[:, fbase:fbase + 8, :], o3)

        # all x loads up front on the (otherwise idle) sync HWDGE ring:
        # loads never queue behind output stores, and chunk c+1's data is
        # in flight while chunk c computes.
        if l1fp8:
            xsb = [xpool.tile([128, 2, CB], FP8, name=f"x8_{c}",
                              tag=f"x8_{c}") for c in range(NCHUNK)]
            if xup:
                for c in range(NCHUNK):
                    nc.sync.dma_start(xsb[c][:],
                                      x8t[:, :, bass.ts(c, CB)])
        else:
            xsb = [[xpool.tile([128, CB], FP16, name=f"x{c}_{k}",
                               tag=f"x{c}_{k}") for k in range(2)]
                   for c in range(NCHUNK)]
            if xup and not mm_only:
                # Chunk 0 as interleaved quarters with its two k-tiles
                # split across BOTH HWDGE rings (k0 on sync behind w1,
                # k1 on scalar behind only vtag+cst32); chunks 1-3 also
                # split across rings.
                Q = CB // 4 if startup_opt else CB // 2
                ring = [nc.sync, nc.scalar]
                for q in range(CB // Q):
                    for k in range(2):
                        r = ring[k] if xring else nc.sync
                        r.dma_start(
                            xsb[0][k][:, q * Q:(q + 1) * Q],
                            xt[k][:, q * Q:(q + 1) * Q])
                for c in range(1, NCHUNK):
                    for k in range(2):
                        ring[k].dma_start(xsb[c][k][:],
                                          xt[k][:, bass.ts(c, CB)])

            if startup_opt and not mm_only:
                # PE warmup during the x-load wait: the For_i barrier +
                # post-drain tail idles the PE for >3.4us every
                # iteration, so HAM re-throttles it to 1.2GHz. A burst
                # of dummy matmuls on resident weights re-opens the
                # 8/8 clock window before the real L1 stream begins.
                wmt = pspool.tile([128, 1024], F32, name="wm", tag="ps",
                                  bufs=3)
                wsrc = w18sb[0][:, 0, :] if l1fp8 else w1all[:]
                wst = w18sb[0][:, 0, 0:128] if l1fp8 else w1all[:, 0:128]
                for i in range(8):
                    nc.tensor.matmul(
                        wmt[:, bass.ts(i % 2, 512)], wst,
                        wsrc[:, bass.ts(i % 2, 512)],
                        start=True, stop=True, skip_group_check=True)

        ev = [0]

        def evac_relu(out_ap, in_ap, bias_ap):
            e = evac_pat[ev[0] % len(evac_pat)]
            ev[0] += 1
            if e == "A":
                nc.scalar.activation(out_ap, in_ap, AF.Relu, bias=bias_ap,
                                     scale=1.0)
            else:
                nc.vector.tensor_scalar(out_ap, in_ap, bias_ap, 0.0,
                                        OP.add, OP.max)

        # Engine queues are strict FIFO: if chunk c's head post ops sat in
        # the ACT/DVE queues ahead of chunk c+1's L1 evacuations, the
        # evacs would stall behind them (their deps resolve only after all
        # of chunk c's PE work), PSUM would fill, and the PE would stall.
        # So POST(c) is emitted one chunk late, after L1(c+1)'s code.
        pending_post = []

        for c in range(NCHUNK if not mm_only else 0):
            if not xup:
                if l1fp8:
                    nc.sync.dma_start(xsb[c][:], x8t[:, :, bass.ts(c, CB)])
                else:
                    for k in range(2):
                        nc.sync.dma_start(xsb[c][k][:],
                                          xt[k][:, bass.ts(c, CB)])

            def layer(nk, wsb, insb, outsb, bsb):
                # per m-tile: NPAIR [128,1024] psum pairs, k-outer for
                # stationary-weight reuse across the 4 nb quarters.
                for m in range(len(outsb)):
                    prs = [pspool.tile([128, 1024], F32, name="ps",
                                       tag="ps", bufs=3)
                           for _ in range(NPAIR)]
                    for k in range(nk):
                        for nb in range(2 * NPAIR):
                            nc.tensor.matmul(
                                prs[nb // 2][:, bass.ts(nb % 2, 512)],
                                wsb[k][:, bass.ts(m, 128)],
                                insb[k][:, bass.ts(nb, 512)],
                                start=(k == 0), stop=(k == nk - 1))
                    for pr in range(NPAIR):
                        evac_relu(outsb[m][:, bass.ts(pr, 1024)],
                                  prs[pr][:], bsb[m][:])

            # L1: f1[m] = relu(W1.T @ x + b1), feature-major fp16
            f1sb = [f1pool.tile([128, CB], FP16, name=f"f1_{m}",
                                tag=f"f1_{m}") for m in range(4)]
            if c == 0 and startup_opt and not l1fp8 and not mm_only:
                # nb-outer for the first chunk: each 512-col block only
                # needs one x quarter, so the PE starts as soon as the
                # first quarter lands instead of waiting for all of x0
                for nb in range(2 * NPAIR):
                    prs = [pspool.tile([128, 1024], F32, name="ps",
                                       tag="ps", bufs=3) for _ in range(2)]
                    for m in range(4):
                        for k in range(2):
                            nc.tensor.matmul(
                                prs[m // 2][:, bass.ts(m % 2, 512)],
                                w1sb[k][:, bass.ts(m, 128)],
                                xsb[0][k][:, bass.ts(nb, 512)],
                                start=(k == 0), stop=(k == 1))
                    for m in range(4):
                        evac_relu(f1sb[m][:, bass.ts(nb, 512)],
                                  prs[m // 2][:, bass.ts(m % 2, 512)],
                                  b1sb[m][:])
            elif l1fp8:
                # fp8 DoubleRow: one matmul contracts all 256 rows; W1 as
                # an exact hi+lo fp8 pair accumulated into the same PSUM
                for m in range(4):
                    prs = [pspool.tile([128, 1024], F32, name="ps",
                                       tag="ps", bufs=3)
                           for _ in range(NPAIR)]
                    for w in range(2):
                        for nb in range(2 * NPAIR):
                            nc.tensor.matmul(
                                prs[nb // 2][:, bass.ts(nb % 2, 512)],
                                w18sb[w][:, :, bass.ts(m, 128)],
                                xsb[c][:, :, bass.ts(nb, 512)],
                                start=(w == 0), stop=(w == 1),
                                perf_mode=PM.DoubleRow)
                    for pr in range(NPAIR):
                        evac_relu(f1sb[m][:, bass.ts(pr, 1024)],
                                  prs[pr][:], b1sb[m][:])
            else:
                layer(2, w1sb, xsb[c], f1sb, b1sb)

            # previous chunk's pass-A post ops go here: their deps are
            # ready by now, and they drain while this chunk's L2 runs.
            if pending_post:
                pending_post.pop(0)()

            # L2: f2[m] = relu(W2.T @ f1 + b2). Custom emission: both
            # m-tiles' pair0 evacs go first (the head's bank 0 reads
            # cols 0:1024 of BOTH f2 tiles), pair1 evacs after.
            f2sb = [f2pool.tile([128, CB], FP16, name=f"f2_{m}",
                                tag=f"f2_{m}") for m in range(2)]
            l2prs = []
            for m in range(2):
                prs = [pspool.tile([128, 1024], F32, name="ps",
                                   tag="ps", bufs=3) for _ in range(NPAIR)]
                for k in range(4):
                    for nb in range(2 * NPAIR):
                        nc.tensor.matmul(
                            prs[nb // 2][:, bass.ts(nb % 2, 512)],
                            w2sb[k][:, bass.ts(m, 128)],
                            f1sb[k][:, bass.ts(nb, 512)],
                            start=(k == 0), stop=(k == 3))
                l2prs.append(prs)
                # pair0 split in two 512-wide evacs: the head's first
                # LDWEIGHTS only needs cols 0:512 of both f2 tiles
                evac_relu(f2sb[m][:, 0:512], prs[0][:, 0:512],
                          b2sb[m][:])
            for m in range(2):
                evac_relu(f2sb[m][:, 512:1024], l2prs[m][0][:, 512:1024],
                          b2sb[m][:])
            for pr in range(1, NPAIR):
                for m in range(2):
                    evac_relu(f2sb[m][:, bass.ts(pr, 1024)],
                              l2prs[m][pr][:], b2sb[m][:])

            # previous chunk's pass-B post ops drain while this chunk's
            # head matmuls run.
            if pending_post:
                pending_post.pop(0)()

            # head, batch-major: bias via K=2 rank-1 matmul (hi/lo fp16
            # rows add bc + 0.5w + C exactly), then per 128-row tile j,
            # f2_tile.T @ Wcat -> [128 batch, 64 steps]; 8 tiles per bank.
            last = (c == NCHUNK - 1)
            st = {}

            def emit_A1(bank, bi, c0=0, c1=512, st=st, zc=None):
                # bank-release half of pass A: masked cummax of raw
                # logits from PSUM (DVE) and the ZA copy to SBUF (DVE).
                # Emitted for BOTH banks before any downstream pass-A
                # work so the PSUM banks free as early as possible.
                # zc=False skips the copy (last chunk: no next consumer)
                zc = za_copy if zc is None else zc
                W = c1 - c0
                M = hpool.tile([128, W], F32, name=f"m_{bi}_{c0}",
                               tag=f"m_{bi}_{c0}")
                nc.vector.tensor_tensor_scan(M[:], masksb[:, c0:c1],
                                             bank[:, c0:c1],
                                             0.0, OP.mult, OP.max)
                ZAc = None
                if zc:
                    # fp16 keeps the Pool-side add single-dtype; the
                    # +C offset costs ~2e-3 logit ulp, well inside the
                    # error budget
                    ZAc = hpool.tile([128, W], FP16, name=f"za_{bi}_{c0}",
                                     tag=f"za_{bi}_{c0}")
                    nc.vector.tensor_copy(ZAc[:], bank[:, c0:c1])
                st[(bi, c0, "a1")] = (M, ZAc)

            def emit_A2(bank, bi, c0=0, c1=512, st=st, zc=None):
                # rest of pass A: tanh((M-C)/2) = 2*p~-1 (ACT), then
                # (w/2)*TM (Pool) and zB = ZA + w*(p~-0.5)
                zc = za_copy if zc is None else zc
                W, g0, g1 = c1 - c0, c0 // T, c1 // T
                M, ZAc = st[(bi, c0, "a1")]
                TM = hpool.tile([128, W], FP16, name=f"sm_{bi}_{c0}",
                                tag=f"sm_{bi}_{c0}")
                nc.scalar.activation(TM[:], M[:], AF.Tanh,
                                     bias=nchsb, scale=0.5)
                TMP = hpool.tile([128, W], FP16, name=f"tp_{bi}_{c0}",
                                 tag=f"tp_{bi}_{c0}")
                t3 = TMP[:].rearrange("p (g t) -> p g t", t=T)
                s3 = TM[:].rearrange("p (g t) -> p g t", t=T)
                nc.gpsimd.tensor_tensor(t3[:, :, 1:], s3[:, :, 0:T - 1],
                                        wg3[:, g0:g1, 1:], OP.mult)
                if zc:
                    # zB on Pool (SBUF-only, single fp16 dtype)
                    ZB = hpool.tile([128, W], FP16, name=f"zb_{bi}_{c0}",
                                    tag=f"zb_{bi}_{c0}")
                    z3 = ZB[:].rearrange("p (g t) -> p g t", t=T)
                    za3 = ZAc[:].rearrange("p (g t) -> p g t", t=T)
                    nc.gpsimd.tensor_tensor(z3[:, :, 1:], t3[:, :, 1:],
                                            za3[:, :, 1:], OP.add)
                else:
                    # zB on DVE reading ZA straight from PSUM
                    ZB = hpool.tile([128, W], F32, name=f"zb_{bi}_{c0}",
                                    tag=f"zb_{bi}_{c0}")
                    z3 = ZB[:].rearrange("p (g t) -> p g t", t=T)
                    p3 = bank[:, c0:c1].rearrange("p (g t) -> p g t", t=T)
                    nc.vector.tensor_tensor(z3[:, :, 1:], t3[:, :, 1:],
                                            p3[:, :, 1:], OP.add)
                # col 0 := M_0, so pass B's sigmoid covers all 64 cols
                # in ONE op (sigmoid(M_0 - C) == pass-A col 0 exactly)
                m3 = M[:].rearrange("p (g t) -> p g t", t=T)
                nc.vector.tensor_copy(z3[:, :, 0:1], m3[:, :, 0:1])
                st[(bi, c0)] = (TM, ZB)

            def emit_A(bank, bi, c0=0, c1=512, st=st, zc=None):
                emit_A1(bank, bi, c0, c1, st, zc)
                emit_A2(bank, bi, c0, c1, st, zc)

            def emit_B(bi, c0=0, c1=512, c=c, st=st):
                # pass B for one window: sigmoids (ACT), masked cummax
                # (DVE), store on the idle sync HWDGE ring
                W = c1 - c0
                TM, ZB = st[(bi, c0)]
                SB = hpool.tile([128, W], FP16, name=f"sb_{bi}_{c0}",
                                tag=f"sb_{bi}_{c0}")
                # one full-width sigmoid: zB col 0 holds M_0, so
                # sigmoid(zB - C) reproduces pass-A's col 0 exactly
                nc.scalar.activation(SB[:], ZB[:], AF.Sigmoid,
                                     bias=ncsb, scale=1.0)
                OUTt = hpool.tile([128, W], FP16, name=f"ou_{bi}_{c0}",
                                  tag=f"ou_{bi}_{c0}")
                # scan is DVE-only (walrus rejects it on Pool)
                nc.vector.tensor_tensor_scan(OUTt[:], masksb[:, c0:c1],
                                             SB[:], 0.0, OP.mult, OP.max)
                o3 = OUTt[:].rearrange("p (g t) -> p g t", t=T)
                fbase = c * (CB // 128) + bi * 8 + c0 // 64
                nc.sync.dma_start(ov[:, fbase:fbase + (W // 64), :],
                                  o3[:, :, :])

            banks = []
            for bi in range(NBANK):
                psw = pspool.tile([128, 512], F32, name="psw", tag="psh",
                                  bufs=2)
                nc.tensor.matmul(psw[:], onsb, brsb,
                                 start=True, stop=False,
                                 skip_group_check=True)
                for j8 in range(8):
                    j = bi * 8 + j8
                    for k in range(2):
                        nc.tensor.matmul(
                            psw[:, bass.ts(j8, T)],
                            f2sb[k][:, bass.ts(j, 128)], wcsb[k],
                            start=False, stop=(k == 1),
                            skip_group_check=True)
                    if (do_post and post_delay and last and tail_split
                            and j8 == 3):
                        # tail: half-bank pass A interleaves with the
                        # remaining head matmuls (shorter drain chain)
                        emit_A(psw, bi, 0, 256, zc=False)
                banks.append(psw)
                if do_post and post_delay and last:
                    if tail_split:
                        emit_A(psw, bi, 256, 512, zc=False)
                    else:
                        emit_A(psw, bi, zc=False)

            if not do_post:
                # bench-only: dump raw logits as "out"
                for bi in range(NBANK):
                    OUTa = hpool.tile([128, 512], FP16, name=f"oq_{bi}",
                                      tag=f"ou_{bi}")
                    nc.vector.tensor_scalar(OUTa[:], banks[bi][:], 0.0, 0.0,
                                            OP.add, OP.max)
                    o3 = OUTa[:].rearrange("p (g t) -> p g t", t=T)
                    fbase = c * (CB // 128) + bi * 8
                    nc.gpsimd.dma_start(ov[:, fbase:fbase + 8, :], o3)
                continue

            def postA(banks=banks):
                # both banks' release ops first, then the slow halves
                for bi in range(NBANK):
                    emit_A1(banks[bi], bi)
                for bi in range(NBANK):
                    emit_A2(banks[bi], bi)

            def postB():
                for bi in range(NBANK):
                    emit_B(bi)

            if not post_delay:
                postA()
                postB()
            elif last:
                # pass A was already emitted per (half-)bank above
                for bi in range(NBANK):
                    if tail_split:
                        emit_B(bi, 0, 256)
                        emit_B(bi, 256, 512)
                    else:
                        emit_B(bi)
            else:
                pending_post.append(postA)
                pending_post.append(postB)

        while pending_post:
            pending_post.pop(0)()

        if loop is not None:
            loop.__exit__(None, None, None)

    nc.compile()
    return nc


def _prep_shared(W1, b1, W2, b2, Wf, bf, Wc, bc):
    fp16 = np.float16
    f32 = np.float32
    W1 = np.asarray(W1, f32)
    W2 = np.asarray(W2, f32)
    Wf = np.asarray(Wf, f32)
    Wc = np.asarray(Wc, f32)
    d = {}
    # k-tiles side by side: [128, k*H]
    d["w1"] = np.ascontiguousarray(np.concatenate(
        [W1[k * 128:(k + 1) * 128, :] for k in range(2)],
        axis=1).astype(fp16))
    # fp8 DoubleRow form: [p, j, m] with contraction row r = j*128 + p,
    # W1 split into an exact hi + lo e4m3 pair
    f8 = mybir.dt.np(FP8)
    w1hi = W1.astype(f8)
    w1lo = (W1 - w1hi.astype(f32)).astype(f8)
    d["w18"] = np.ascontiguousarray(np.stack([
        w.reshape(2, 128, H1).transpose(1, 0, 2) for w in (w1hi, w1lo)]))
    d["w2"] = np.ascontiguousarray(np.concatenate(
        [W2[k * 128:(k + 1) * 128, :] for k in range(4)],
        axis=1).astype(fp16))
    wcat = np.concatenate([Wf, Wc[:, :H2].T], axis=1)   # [256, 64]
    d["wcat"] = np.ascontiguousarray(np.concatenate(
        [wcat[k * 128:(k + 1) * 128, :] for k in range(2)],
        axis=1).astype(fp16))
    # [128, 8] f32: b1 m-tiles | b2 m-tiles | -C | -C/2
    cst = np.zeros((128, 8), f32)
    cst[:, 0:4] = np.asarray(b1, f32).reshape(4, 128).T
    cst[:, 4:6] = np.asarray(b2, f32).reshape(2, 128).T
    cst[:, 6] = -COFF
    cst[:, 7] = -0.5 * COFF
    d["cst32"] = cst
    bcat = np.concatenate([np.asarray(bf, f32), np.asarray(bc, f32)])
    wprev = Wc[:, H2]                                   # [63]
    wrow = np.concatenate([np.zeros(1, f32), wprev])    # [64], 0 at t=0
    # pass A uses constant prev=0.5: fold 0.5*w_t (plus the positivity
    # offset C) into the head bias, applied as an exact hi/lo fp16 pair
    brow = np.tile(bcat + 0.5 * wrow + COFF, 8).astype(f32)   # [512]
    bhi = brow.astype(fp16)
    blo = (brow - bhi.astype(f32)).astype(fp16)
    # [2, 640]: bias hi/lo rows | ones
    d["bro"] = np.ascontiguousarray(np.concatenate(
        [np.stack([bhi, blo]), np.ones((2, 128), fp16)], axis=1))
    # [128, 1024]: scan reset mask | pre-halved w row ((w/2) * TM trick)
    mask = np.ones((128, 512), fp16)
    mask[:, 0::T] = 0.0
    d["mwg"] = np.ascontiguousarray(np.concatenate(
        [mask, np.tile(0.5 * wrow, (128, 8)).astype(fp16)], axis=1))
    return d


def _core_inputs(x, shared, c):
    fp16 = np.float16
    xs = x[c * BL:(c + 1) * BL, :]
    m = dict(shared)
    xT = np.ascontiguousarray(xs.T)                      # [256, BL] f32
    m["xt"] = xT.astype(fp16).reshape(2, 128, BL)
    m["x8t"] = np.ascontiguousarray(
        xT.astype(mybir.dt.np(FP8)).reshape(2, 128, BL).transpose(1, 0, 2))
    return m


def kernel(x, W1, b1, W2, b2, Wf, bf, Wc, bc):
    if "nc" not in _CACHE:
        _CACHE["nc"] = _build()
    nc = _CACHE["nc"]

    x = np.asarray(x, np.float32)
    shared = _prep_shared(W1, b1, W2, b2, Wf, bf, Wc, bc)
    in_maps = [_core_inputs(x, shared, c) for c in range(NCORES)]

    # zero-fill any declared inputs we don't feed (e.g. the variant tag)
    pname = nc.partition_id_tensor.name if nc.partition_id_tensor else None
    for alloc in nc.m.functions[0].allocations:
        if (isinstance(alloc, mybir.MemoryLocationSet)
                and alloc.kind == "ExternalInput"):
            nm = alloc.memorylocations[0].name
            if nm != pname:
                for m in in_maps:
                    if nm not in m:
                        m[nm] = np.zeros(tuple(alloc.tensor_shape),
                                         mybir.dt.np(alloc.dtype))

    res = run_bass_kernel_spmd(nc, in_maps, list(range(NCORES)))
    outs = [np.asarray(res.results[c]["out"], np.float32)
            for c in range(NCORES)]
    return np.concatenate(outs, axis=0)
